# revision 7
# baseline (speedup 1.0000x reference)
"""Graph-transformer (nn_GTModel) Trainium2 kernel, 8-core SPMD, multi-launch.

Device work is split into 4 NEFFs (collectives are unavailable on this
runtime, so the tiny cross-core exchanges go through the host):
  A  : embedding       -> h0^T (full, bf16) + local h0 (fp32)
  B1 : attention block -> X = O-proj(attn) + h (local, fp32) + bn1 partial sums
  B2 : bn1-apply + FFN -> y = h1 + ffn(h1)  (local, fp32) + bn2 partial sums
  C  : prediction head -> out rows (local)
Host between launches: sum 4KB batchnorm partials, compute scale/shift,
apply the bn2 affine to assemble the next layer's full hidden state, and
reshard. All matmuls / softmax / elementwise heavy work is on device.

Layout/tricks:
- Feature-major activations hT [512 f, n]; node rows sharded 8 ways.
- Host permutes Wq/Wk/Wv cols (Wo rows) so heads are contiguous;
  attention scale folded into Wq/bq; V bias folded into O-proj bias.
- Scores computed transposed (S^T [m, n]): softmax denominator comes
  free from an appended ones-column on node-major V in the attn@V matmul.
- bf16 for TensorE-facing tensors, fp32 accumulation/residual/stats.
"""

import numpy as np
import ml_dtypes

import concourse.bacc as bacc
import concourse.mybir as mybir
import concourse.tile as tile
from concourse.bass_utils import run_bass_kernel_spmd

NCORES = 8
N = 4096
NL = N // NCORES
F = 512
H = 8
D = 64
FF = 1024
IN_DIM = 1024
NCL = 20
EPS = 1e-5
SCALING = float(F) ** -0.5
FT = F // 128
MB = N // 128

bf16 = ml_dtypes.bfloat16
f32 = mybir.dt.float32
bf = mybir.dt.bfloat16

AX = mybir.AxisListType.X
MUL = mybir.AluOpType.mult
ADD = mybir.AluOpType.add
SUB = mybir.AluOpType.subtract


def _act_fn(*names):
    t = mybir.ActivationFunctionType
    for n in names:
        if hasattr(t, n):
            return getattr(t, n)
    raise AttributeError(names)


def _new_nc():
    return bacc.Bacc("TRN2", target_bir_lowering=False, debug=False, num_devices=NCORES)


def _bias_tile(nc, pool, dram, nfeat, tag):
    """Load a [1, nfeat] DRAM vector as per-partition [128, nfeat//128]."""
    t = pool.tile([128, nfeat // 128], f32, tag=tag)
    nc.sync.dma_start(t[:], dram[:, :].rearrange("o (f p) -> p (o f)", p=128))
    return t


def _bias_row(nc, pool, dram, nfeat, tag):
    t = pool.tile([1, nfeat], bf, tag=tag, name=tag)
    nc.sync.dma_start(t[:], dram[:, :])
    return t


def _load_w(nc, pool, dram, rows, cols, tag):
    tiles = []
    for fi in range(rows // 128):
        t = pool.tile([128, cols], bf, tag=f"{tag}{fi}", name=f"{tag}{fi}")
        nc.sync.dma_start(t[:], dram[fi * 128 : (fi + 1) * 128, :])
        tiles.append(t)
    return tiles


def _linT(nc, ps_pool, dst, w_tiles, src, bias_row, ones_row, act, n_cols):
    """dst[fo][:, :n_cols] = act(sum_fi w[fi][:, fo].T @ src[fi] + bias[fo]).

    bias_row: [1, F_out] bf16 SBUF row; folded in as a K=1 matmul."""
    nfi = len(w_tiles)
    for fo in range(len(dst)):
        for ck in range((n_cols + 511) // 512):
            c0, c1 = ck * 512, min((ck + 1) * 512, n_cols)
            ps = ps_pool.tile([128, 512], f32, tag="ps", name="ps")
            if bias_row is not None:
                nc.tensor.matmul(
                    ps[:, : c1 - c0],
                    bias_row[:, fo * 128 : (fo + 1) * 128],
                    ones_row[:, : c1 - c0],
                    start=True, stop=False,
                )
            for fi in range(nfi):
                nc.tensor.matmul(
                    ps[:, : c1 - c0],
                    w_tiles[fi][:, fo * 128 : (fo + 1) * 128],
                    src[fi][:, c0:c1],
                    start=(bias_row is None and fi == 0),
                    stop=(fi == nfi - 1),
                )
            nc.scalar.activation(dst[fo][:, c0:c1], ps[:, : c1 - c0], act)


def build_A():
    """Embedding: h0^T full (bf16) + h0 local (fp32)."""
    nc = _new_nc()
    COPY = _act_fn("Copy", "Identity", "copy")
    featT = nc.dram_tensor("featT", [IN_DIM, N], bf, kind="ExternalInput")
    featTl = nc.dram_tensor("featTl", [IN_DIM, NL], bf, kind="ExternalInput")
    wemb_d = nc.dram_tensor("wemb", [IN_DIM, F], bf, kind="ExternalInput")
    bemb_d = nc.dram_tensor("bemb", [1, F], f32, kind="ExternalInput")
    bembr_d = nc.dram_tensor("bembr", [1, F], bf, kind="ExternalInput")
    h0T_d = nc.dram_tensor("h0T", [F, N], bf, kind="ExternalOutput")
    h0l_d = nc.dram_tensor("h0l", [F, NL], f32, kind="ExternalOutput")

    with tile.TileContext(nc) as tc:
        with (
            tc.tile_pool(name="sb", bufs=2) as sb,
            tc.tile_pool(name="st", bufs=3) as st,
            tc.tile_pool(name="psum", bufs=3, space="PSUM") as psp,
        ):
            wemb = _load_w(nc, sb, wemb_d, IN_DIM, F, "wemb")
            bemb = _bias_tile(nc, sb, bemb_d, F, "bemb")
            bembr = _bias_row(nc, sb, bembr_d, F, "bembr")
            ones_row = sb.tile([1, 512], bf, tag="ones_row", name="ones_row")
            nc.gpsimd.memset(ones_row[:], 1.0)
            for ck in range(N // 512):
                fts = []
                for fi in range(IN_DIM // 128):
                    t = st.tile([128, 512], bf, tag=f"ftf{fi % 4}", name=f"ftf{fi % 4}")
                    nc.sync.dma_start(
                        t[:], featT[fi * 128 : (fi + 1) * 128, ck * 512 : (ck + 1) * 512]
                    )
                    fts.append(t)
                for fo in range(FT):
                    ps = psp.tile([128, 512], f32, tag="ps", name="ps")
                    nc.tensor.matmul(
                        ps[:], bembr[:, fo * 128 : (fo + 1) * 128], ones_row[:],
                        start=True, stop=False,
                    )
                    for fi in range(IN_DIM // 128):
                        nc.tensor.matmul(
                            ps[:], wemb[fi][:, fo * 128 : (fo + 1) * 128], fts[fi][:],
                            start=False, stop=(fi == IN_DIM // 128 - 1),
                        )
                    ot = st.tile([128, 512], bf, tag="ot", name="ot")
                    nc.scalar.activation(ot[:], ps[:], COPY)
                    nc.sync.dma_start(
                        h0T_d[fo * 128 : (fo + 1) * 128, ck * 512 : (ck + 1) * 512], ot[:]
                    )
            ftl = []
            for fi in range(IN_DIM // 128):
                t = st.tile([128, NL], bf, tag=f"ftl{fi}", name=f"ftl{fi}")
                nc.sync.dma_start(t[:], featTl[fi * 128 : (fi + 1) * 128, :])
                ftl.append(t)
            for fo in range(FT):
                ps = psp.tile([128, 512], f32, tag="ps", name="ps")
                for fi in range(IN_DIM // 128):
                    nc.tensor.matmul(
                        ps[:], wemb[fi][:, fo * 128 : (fo + 1) * 128], ftl[fi][:],
                        start=(fi == 0), stop=(fi == IN_DIM // 128 - 1),
                    )
                ol = st.tile([128, NL], f32, tag="ol", name="ol")
                nc.vector.tensor_scalar(ol[:], ps[:], bemb[:, fo : fo + 1], None, op0=ADD)
                nc.sync.dma_start(h0l_d[fo * 128 : (fo + 1) * 128, :], ol[:])
    nc.compile()
    return nc


def build_B1():
    """Attention block: X = O-proj(attn(h)) + h, bn1 partial sums."""
    nc = _new_nc()
    EXP = _act_fn("Exp", "exp")
    COPY = _act_fn("Copy", "Identity", "copy")
    SQUARE = _act_fn("Square", "square")

    hT_d = nc.dram_tensor("hT", [F, N], bf, kind="ExternalInput")
    hl_d = nc.dram_tensor("hl", [F, NL], f32, kind="ExternalInput")
    AT_d = nc.dram_tensor("AT", [N, NL], bf, kind="ExternalInput")
    wq_d = nc.dram_tensor("wq", [F, F], bf, kind="ExternalInput")
    wk_d = nc.dram_tensor("wk", [F, F], bf, kind="ExternalInput")
    wv_d = nc.dram_tensor("wv", [F, F], bf, kind="ExternalInput")
    wo_d = nc.dram_tensor("wo", [F, F], bf, kind="ExternalInput")
    bq_d = nc.dram_tensor("bq", [1, F], bf, kind="ExternalInput")
    bk_d = nc.dram_tensor("bk", [1, F], bf, kind="ExternalInput")
    bo2_d = nc.dram_tensor("bo2", [1, F], f32, kind="ExternalInput")
    X_d = nc.dram_tensor("X", [F, NL], f32, kind="ExternalOutput")
    p1_d = nc.dram_tensor("p1", [128, 8], f32, kind="ExternalOutput")

    with tile.TileContext(nc) as tc:
        with (
            tc.tile_pool(name="pp", bufs=1) as pp,
            tc.tile_pool(name="wp", bufs=1) as wp,
            tc.tile_pool(name="st", bufs=3) as st,
            tc.tile_pool(name="ep", bufs=3) as epo,
            tc.tile_pool(name="psA", bufs=3, space="PSUM") as psA,
            tc.tile_pool(name="psB", bufs=3, space="PSUM") as psB,
            tc.tile_pool(name="psO", bufs=2, space="PSUM") as psOp,
        ):
            AT = [pp.tile([128, NL], bf, tag=f"AT{m}", name=f"AT{m}") for m in range(MB)]
            for m in range(MB):
                nc.sync.dma_start(AT[m][:], AT_d[m * 128 : (m + 1) * 128, :])
            hT = [pp.tile([128, N], bf, tag=f"hT{t}", name=f"hT{t}") for t in range(FT)]
            for t in range(FT):
                nc.sync.dma_start(hT[t][:], hT_d[t * 128 : (t + 1) * 128, :])
            hc = [pp.tile([128, NL], f32, tag=f"hc{t}", name=f"hc{t}") for t in range(FT)]
            hTl = [pp.tile([128, NL], bf, tag=f"hTl{t}", name=f"hTl{t}") for t in range(FT)]
            for t in range(FT):
                nc.sync.dma_start(hc[t][:], hl_d[t * 128 : (t + 1) * 128, :])
                nc.vector.tensor_copy(hTl[t][:], hc[t][:])
            KT = [pp.tile([128, N], bf, tag=f"KT{t}", name=f"KT{t}") for t in range(FT)]
            Vn = [pp.tile([128, H * (D + 1)], bf, tag=f"Vn{m}", name=f"Vn{m}") for m in range(MB)]
            QT = [pp.tile([128, NL], bf, tag=f"QT{t}", name=f"QT{t}") for t in range(FT)]
            OT = [pp.tile([128, NL], bf, tag=f"OT{t}", name=f"OT{t}") for t in range(FT)]
            ones64 = pp.tile([1, 64], bf, tag="ones64", name="ones64")
            nc.gpsimd.memset(ones64[:], 1.0)

            wq = _load_w(nc, wp, wq_d, F, F, "wq")
            wk = _load_w(nc, wp, wk_d, F, F, "wk")
            wv = _load_w(nc, wp, wv_d, F, F, "wv")
            wo = _load_w(nc, wp, wo_d, F, F, "wo")
            bq = _bias_row(nc, wp, bq_d, F, "bq")
            bk = _bias_row(nc, wp, bk_d, F, "bk")
            bo2 = _bias_tile(nc, wp, bo2_d, F, "bo2")
            ones_row = wp.tile([1, 512], bf, tag="ones_row", name="ones_row")
            nc.gpsimd.memset(ones_row[:], 1.0)

            _linT(nc, psB, KT, wk, hT, bk, ones_row, COPY, N)
            _linT(nc, psB, QT, wq, hTl, bq, ones_row, COPY, NL)
            for m in range(MB):
                nc.gpsimd.memset(Vn[m][:], 1.0)
                ps = psB.tile([128, 512], f32, tag="ps", name="ps")
                for fi in range(FT):
                    nc.tensor.matmul(
                        ps[:], hT[fi][:, m * 128 : (m + 1) * 128], wv[fi][:],
                        start=(fi == 0), stop=(fi == FT - 1),
                    )
                nc.vector.tensor_copy(
                    Vn[m][:].rearrange("p (h x) -> p (h x)", x=D + 1)
                    .rearrange("p (h x) -> p h x", x=D + 1)[:, :, :D],
                    ps[:].rearrange("p (h d) -> p h d", d=D),
                )

            for h in range(H):
                ft, po = h // 2, 64 * (h % 2)
                pO = psOp.tile([128, NL], f32, tag="pO", name="pO")
                epair = None
                for m in range(MB):
                    pS = psA.tile([128, NL], f32, tag="pS", name="pS")
                    nc.tensor.matmul(
                        pS[:],
                        KT[ft][po : po + 64, m * 128 : (m + 1) * 128],
                        QT[ft][po : po + 64, :],
                        start=True, stop=True,
                    )
                    if m % 2 == 0:
                        epair = epo.tile([128, 2 * NL], bf, tag="epair", name="epair")
                    nc.vector.tensor_tensor(
                        epair[:, (m % 2) * NL : (m % 2 + 1) * NL], pS[:], AT[m][:], op=MUL
                    )
                    if m % 2 == 1:
                        nc.scalar.activation(epair[:], epair[:], EXP)
                        for mm in (m - 1, m):
                            nc.tensor.matmul(
                                pO[: D + 1, :],
                                Vn[mm][:, h * (D + 1) : (h + 1) * (D + 1)],
                                epair[:, (mm % 2) * NL : (mm % 2 + 1) * NL],
                                start=(mm == 0), stop=(mm == MB - 1),
                            )
                rs = st.tile([1, NL], bf, tag="rs", name="rs")
                with nc.allow_low_precision(reason="softmax denom recip in bf16"):
                    nc.vector.reciprocal(rs[:], pO[D : D + 1, :])
                pR = psA.tile([128, NL], f32, tag="pS", name="pS")
                nc.tensor.matmul(pR[:64, :], ones64[:], rs[:], start=True, stop=True)
                rb = st.tile([64, NL], bf, tag="rb", name="rb")
                nc.vector.tensor_copy(rb[:], pR[:64, :])
                nc.vector.tensor_tensor(
                    OT[ft][po : po + 64, :], pO[:D, :], rb[:], op=MUL
                )

            part = st.tile([128, 8], f32, tag="part", name="part")
            sq = st.tile([128, NL], f32, tag="sq", name="sq")
            for fo in range(FT):
                nc.vector.tensor_scalar(
                    hc[fo][:], hc[fo][:], bo2[:, fo : fo + 1], None, op0=ADD
                )
                ps = psB.tile([128, 512], f32, tag="ps", name="ps")
                for fi in range(FT):
                    nc.tensor.matmul(
                        ps[:], wo[fi][:, fo * 128 : (fo + 1) * 128], OT[fi][:],
                        start=(fi == 0), stop=(fi == FT - 1),
                    )
                Xt = st.tile([128, NL], f32, tag="Xt", name="Xt")
                nc.vector.tensor_tensor(Xt[:], ps[:], hc[fo][:], op=ADD)
                nc.sync.dma_start(X_d[fo * 128 : (fo + 1) * 128, :], Xt[:])
                nc.vector.tensor_reduce(
                    part[:, 2 * fo : 2 * fo + 1], Xt[:], axis=AX, op=ADD
                )
                nc.scalar.activation(sq[:], Xt[:], SQUARE)
                nc.vector.tensor_reduce(
                    part[:, 2 * fo + 1 : 2 * fo + 2], sq[:], axis=AX, op=ADD
                )
            nc.sync.dma_start(p1_d[:, :], part[:])
    nc.compile()
    return nc


def build_B2():
    """bn1-apply + FFN + residual: y, bn2 partial sums."""
    nc = _new_nc()
    RELU = _act_fn("Relu", "relu")
    SQUARE = _act_fn("Square", "square")
    X_di = nc.dram_tensor("X", [F, NL], f32, kind="ExternalInput")
    s1_d = nc.dram_tensor("s1", [128, FT], f32, kind="ExternalInput")
    t1_d = nc.dram_tensor("t1", [128, FT], f32, kind="ExternalInput")
    w1_d = nc.dram_tensor("w1", [F, FF], bf, kind="ExternalInput")
    w2_d = nc.dram_tensor("w2", [FF, F], bf, kind="ExternalInput")
    b1f_d = nc.dram_tensor("b1f", [1, FF], bf, kind="ExternalInput")
    b2f_d = nc.dram_tensor("b2f", [1, F], f32, kind="ExternalInput")
    y_d = nc.dram_tensor("y", [F, NL], f32, kind="ExternalOutput")
    p2_d = nc.dram_tensor("p2", [128, 8], f32, kind="ExternalOutput")

    with tile.TileContext(nc) as tc:
        with (
            tc.tile_pool(name="sb", bufs=1) as sb,
            tc.tile_pool(name="st", bufs=3) as st,
            tc.tile_pool(name="psum", bufs=4, space="PSUM") as psp,
        ):
            w1 = _load_w(nc, sb, w1_d, F, FF, "w1")
            w2 = _load_w(nc, sb, w2_d, FF, F, "w2")
            b1f = _bias_row(nc, sb, b1f_d, FF, "b1f")
            b2f = _bias_tile(nc, sb, b2f_d, F, "b2f")
            ones_row = sb.tile([1, 512], bf, tag="ones_row", name="ones_row")
            nc.gpsimd.memset(ones_row[:], 1.0)
            s1 = sb.tile([128, FT], f32, tag="s1", name="s1")
            t1 = sb.tile([128, FT], f32, tag="t1", name="t1")
            nc.sync.dma_start(s1[:], s1_d[:, :])
            nc.sync.dma_start(t1[:], t1_d[:, :])
            h1 = [sb.tile([128, NL], f32, tag=f"h1_{t}", name=f"h1_{t}") for t in range(FT)]
            h1b = [sb.tile([128, NL], bf, tag=f"h1b{t}", name=f"h1b{t}") for t in range(FT)]
            for fo in range(FT):
                Xt = st.tile([128, NL], f32, tag="Xt", name="Xt")
                nc.sync.dma_start(Xt[:], X_di[fo * 128 : (fo + 1) * 128, :])
                nc.vector.tensor_scalar(
                    h1[fo][:], Xt[:], s1[:, fo : fo + 1], t1[:, fo : fo + 1],
                    op0=MUL, op1=ADD,
                )
                nc.vector.tensor_copy(h1b[fo][:], h1[fo][:])
            r16 = [sb.tile([128, NL], bf, tag=f"r16_{g}", name=f"r16_{g}") for g in range(FF // 128)]
            _linT(nc, psp, r16, w1, h1b, b1f, ones_row, RELU, NL)
            part = st.tile([128, 8], f32, tag="part", name="part")
            sq = st.tile([128, NL], f32, tag="sq", name="sq")
            for fo in range(FT):
                nc.vector.tensor_scalar(
                    h1[fo][:], h1[fo][:], b2f[:, fo : fo + 1], None, op0=ADD
                )
                ps = psp.tile([128, 512], f32, tag="ps", name="ps")
                for gi in range(FF // 128):
                    nc.tensor.matmul(
                        ps[:], w2[gi][:, fo * 128 : (fo + 1) * 128], r16[gi][:],
                        start=(gi == 0), stop=(gi == FF // 128 - 1),
                    )
                yt = st.tile([128, NL], f32, tag="yt", name="yt")
                nc.vector.tensor_tensor(yt[:], ps[:], h1[fo][:], op=ADD)
                nc.sync.dma_start(y_d[fo * 128 : (fo + 1) * 128, :], yt[:])
                SQ = _act_fn("Square", "square")
                nc.vector.tensor_reduce(
                    part[:, 2 * fo : 2 * fo + 1], yt[:], axis=AX, op=ADD
                )
                nc.scalar.activation(sq[:], yt[:], SQ)
                nc.vector.tensor_reduce(
                    part[:, 2 * fo + 1 : 2 * fo + 2], sq[:], axis=AX, op=ADD
                )
            nc.sync.dma_start(p2_d[:, :], part[:])
    nc.compile()
    return nc


def build_C():
    """Prediction head on local rows."""
    nc = _new_nc()
    RELU = _act_fn("Relu", "relu")
    hl_d = nc.dram_tensor("hl", [F, NL], f32, kind="ExternalInput")
    wp1_d = nc.dram_tensor("wp1", [F, F // 2], bf, kind="ExternalInput")
    bp1_d = nc.dram_tensor("bp1", [1, F // 2], bf, kind="ExternalInput")
    wp2_d = nc.dram_tensor("wp2", [F // 2, NCL], bf, kind="ExternalInput")
    bp2_d = nc.dram_tensor("bp2", [1, NCL], f32, kind="ExternalInput")
    ident_d = nc.dram_tensor("ident", [128, 128], f32, kind="ExternalInput")
    out_d = nc.dram_tensor("out", [NL, NCL], f32, kind="ExternalOutput")
    with tile.TileContext(nc) as tc:
        with (
            tc.tile_pool(name="sb", bufs=1) as sb,
            tc.tile_pool(name="st", bufs=3) as st,
            tc.tile_pool(name="psum", bufs=3, space="PSUM") as psp,
        ):
            hTl = [sb.tile([128, NL], bf, tag=f"hTl{t}", name=f"hTl{t}") for t in range(FT)]
            for t in range(FT):
                hf = st.tile([128, NL], f32, tag="hf", name="hf")
                nc.sync.dma_start(hf[:], hl_d[t * 128 : (t + 1) * 128, :])
                nc.vector.tensor_copy(hTl[t][:], hf[:])
            wp1 = _load_w(nc, sb, wp1_d, F, F // 2, "wp1")
            bp1 = _bias_row(nc, sb, bp1_d, F // 2, "bp1")
            ones_row = sb.tile([1, 512], bf, tag="ones_row", name="ones_row")
            nc.gpsimd.memset(ones_row[:], 1.0)
            wp2 = _load_w(nc, sb, wp2_d, F // 2, NCL, "wp2")
            bp2 = sb.tile([NCL, 1], f32, tag="bp2", name="bp2")
            nc.sync.dma_start(bp2[:], bp2_d[:, :].rearrange("o p -> p o"))
            ident = sb.tile([128, 128], f32, tag="ident", name="ident")
            nc.sync.dma_start(ident[:], ident_d[:, :])
            p1 = [sb.tile([128, NL], bf, tag=f"p1_{t}", name=f"p1_{t}") for t in range(2)]
            _linT(nc, psp, p1, wp1, hTl, bp1, ones_row, RELU, NL)
            ps = psp.tile([NCL, NL], f32, tag="pred", name="pred", bufs=1)
            for pi in range(2):
                nc.tensor.matmul(
                    ps[:], wp2[pi][:, :], p1[pi][:], start=(pi == 0), stop=(pi == 1)
                )
            outT = st.tile([NCL, NL], f32, tag="outT", name="outT")
            nc.vector.tensor_scalar(outT[:], ps[:], bp2[:, :], None, op0=ADD)
            for ck in range(NL // 128):
                pt = psp.tile([128, NCL], f32, tag="pt", name="pt", bufs=2)
                nc.tensor.matmul(
                    pt[:], outT[:, ck * 128 : (ck + 1) * 128], ident[:NCL, :NCL],
                    start=True, stop=True, is_transpose=True,
                )
                ot = st.tile([128, NCL], f32, tag="ot", name="ot")
                nc.vector.tensor_copy(ot[:], pt[:])
                nc.sync.dma_start(out_d[ck * 128 : (ck + 1) * 128, :], ot[:])
    nc.compile()
    return nc


_CACHE = {}


def _get(name, builder):
    if name not in _CACHE:
        _CACHE[name] = builder()
    return _CACHE[name]


def _run(nc, in_maps, trace=False):
    return run_bass_kernel_spmd(
        nc, in_maps, core_ids=list(range(NCORES)), trace=trace
    )


def _bn_host(parts, g, b):
    """parts: list of [128, 8] per core -> s,t as [128, 4] per-partition."""
    tot = np.sum(np.stack(parts, 0), axis=0)  # [128, 8]
    s = tot.reshape(128, 4, 2)[:, :, 0]  # sums,  feature f = fo*128 + p
    ss = tot.reshape(128, 4, 2)[:, :, 1]
    mu = s / N
    var = ss / N - mu * mu
    gpf = g.reshape(4, 128).T  # [p, fo]
    bpf = b.reshape(4, 128).T
    sc = gpf / np.sqrt(var + EPS)
    sh = bpf - mu * sc
    return sc.astype(np.float32), sh.astype(np.float32), mu, var


def _bn_apply_full(yT_full, sc, sh):
    """yT_full [512, 4096]; sc/sh [128, 4] (p, fo) -> per-feature."""
    scf = sc.T.reshape(F, 1)  # feature f = fo*128+p -> row-major (fo, p)
    shf = sh.T.reshape(F, 1)
    return yT_full * scf + shf


def kernel(A, features, params, trace=False, timing=None):
    A = np.asarray(A, np.float32)
    features = np.asarray(features, np.float32)
    perm = np.array([d * H + h for h in range(H) for d in range(D)])

    W = lambda p: np.asarray(p["w"], np.float32)
    Bv = lambda p: np.asarray(p["b"], np.float32)

    featT = np.ascontiguousarray(features.T).astype(bf16)
    AT = np.ascontiguousarray(A.T).astype(bf16)

    def _t(res):
        if timing is not None and res.exec_time_ns:
            timing.append(res.exec_time_ns)

    # ---- A: embedding ----
    base = {
        "featT": featT,
        "wemb": W(params["emb"]).astype(bf16),
        "bemb": Bv(params["emb"]).reshape(1, F).astype(np.float32),
        "bembr": Bv(params["emb"]).reshape(1, F).astype(bf16),
    }
    maps = []
    for c in range(NCORES):
        m = dict(base)
        m["featTl"] = np.ascontiguousarray(featT[:, c * NL : (c + 1) * NL])
        maps.append(m)
    resA = _run(_get("A", build_A), maps, trace)
    _t(resA)
    hT_full = resA.results[0]["h0T"]  # bf16 [F, N]
    hl = [resA.results[c]["h0l"] for c in range(NCORES)]  # f32 [F, NL]

    ncB1 = _get("B1", build_B1)
    ncB2 = _get("B2", build_B2)
    for l, lp in enumerate(params["layers"]):
        wq = (W(lp["q"])[:, perm] * SCALING).astype(bf16)
        bq = (Bv(lp["q"])[perm] * SCALING).reshape(1, F).astype(bf16)
        wk = W(lp["k"])[:, perm].astype(bf16)
        bk = Bv(lp["k"])[perm].reshape(1, F).astype(bf16)
        wv = W(lp["v"])[:, perm].astype(bf16)
        wo = W(lp["o"])[perm, :].astype(bf16)
        bo2 = (Bv(lp["v"]) @ W(lp["o"]) + Bv(lp["o"])).reshape(1, F).astype(np.float32)
        maps = []
        for c in range(NCORES):
            maps.append({
                "hT": np.ascontiguousarray(hT_full),
                "hl": np.ascontiguousarray(hl[c]),
                "AT": np.ascontiguousarray(AT[:, c * NL : (c + 1) * NL]),
                "wq": wq, "bq": bq, "wk": wk, "bk": bk, "wv": wv,
                "wo": wo, "bo2": bo2,
            })
        r1 = _run(ncB1, maps, trace)
        _t(r1)
        Xs = [r1.results[c]["X"] for c in range(NCORES)]
        parts1 = [r1.results[c]["p1"] for c in range(NCORES)]
        s1, t1, _, _ = _bn_host(parts1, np.asarray(lp["g1"], np.float32),
                                np.asarray(lp["b1"], np.float32))
        maps = []
        w1 = W(lp["ffn1"]).astype(bf16)
        w2 = W(lp["ffn2"]).astype(bf16)
        b1f = Bv(lp["ffn1"]).reshape(1, FF).astype(bf16)
        b2f = Bv(lp["ffn2"]).reshape(1, F).astype(np.float32)
        for c in range(NCORES):
            maps.append({
                "X": Xs[c], "s1": s1, "t1": t1,
                "w1": w1, "w2": w2, "b1f": b1f, "b2f": b2f,
            })
        r2 = _run(ncB2, maps, trace)
        _t(r2)
        ys = [r2.results[c]["y"] for c in range(NCORES)]
        parts2 = [r2.results[c]["p2"] for c in range(NCORES)]
        s2, t2, _, _ = _bn_host(parts2, np.asarray(lp["g2"], np.float32),
                                np.asarray(lp["b2"], np.float32))
        yT_full = np.concatenate(ys, axis=1)  # [F, N] f32
        hT_next = _bn_apply_full(yT_full.astype(np.float32), s2, t2)
        hT_full = hT_next.astype(bf16)
        hl = [np.ascontiguousarray(hT_next[:, c * NL : (c + 1) * NL]).astype(np.float32)
              for c in range(NCORES)]

    # ---- C: prediction head ----
    maps = []
    basec = {
        "wp1": W(params["pred1"]).astype(bf16),
        "bp1": Bv(params["pred1"]).reshape(1, F // 2).astype(bf16),
        "wp2": W(params["pred2"]).astype(bf16),
        "bp2": Bv(params["pred2"]).reshape(1, NCL).astype(np.float32),
        "ident": np.eye(128, dtype=np.float32),
    }
    for c in range(NCORES):
        m = dict(basec)
        m["hl"] = hl[c]
        maps.append(m)
    resC = _run(_get("C", build_C), maps, trace)
    _t(resC)
    out = np.concatenate([resC.results[c]["out"] for c in range(NCORES)], axis=0)
    return np.asarray(out, np.float32)


# revision 8
# speedup vs baseline: 1.0278x; 1.0278x over previous
"""Graph-transformer (nn_GTModel) Trainium2 kernel, 8-core SPMD, multi-launch.

Device work is split into 4 NEFFs (collectives are unavailable on this
runtime, so the tiny cross-core exchanges go through the host):
  A  : embedding       -> h0^T (full, bf16) + local h0 (fp32)
  B1 : attention block -> X = O-proj(attn) + h (local, fp32) + bn1 partial sums
  B2 : bn1-apply + FFN -> y = h1 + ffn(h1)  (local, fp32) + bn2 partial sums
  C  : prediction head -> out rows (local)
Host between launches: sum 4KB batchnorm partials, compute scale/shift,
apply the bn2 affine to assemble the next layer's full hidden state, and
reshard. All matmuls / softmax / elementwise heavy work is on device.

Layout/tricks:
- Feature-major activations hT [512 f, n]; node rows sharded 8 ways.
- Host permutes Wq/Wk/Wv cols (Wo rows) so heads are contiguous;
  attention scale folded into Wq/bq; V bias folded into O-proj bias.
- Scores computed transposed (S^T [m, n]): softmax denominator comes
  free from an appended ones-column on node-major V in the attn@V matmul.
- bf16 for TensorE-facing tensors, fp32 accumulation/residual/stats.
"""

import numpy as np
import ml_dtypes

import concourse.bacc as bacc
import concourse.mybir as mybir
import concourse.tile as tile
from concourse.bass_utils import run_bass_kernel_spmd

NCORES = 8
N = 4096
NL = N // NCORES
F = 512
H = 8
D = 64
FF = 1024
IN_DIM = 1024
NCL = 20
EPS = 1e-5
SCALING = float(F) ** -0.5
FT = F // 128
MB = N // 128

bf16 = ml_dtypes.bfloat16
f32 = mybir.dt.float32
bf = mybir.dt.bfloat16

AX = mybir.AxisListType.X
MUL = mybir.AluOpType.mult
ADD = mybir.AluOpType.add
SUB = mybir.AluOpType.subtract


def _act_fn(*names):
    t = mybir.ActivationFunctionType
    for n in names:
        if hasattr(t, n):
            return getattr(t, n)
    raise AttributeError(names)


def _new_nc():
    return bacc.Bacc("TRN2", target_bir_lowering=False, debug=False, num_devices=NCORES)


def _bias_tile(nc, pool, dram, nfeat, tag):
    """Load a [1, nfeat] DRAM vector as per-partition [128, nfeat//128]."""
    t = pool.tile([128, nfeat // 128], f32, tag=tag)
    nc.sync.dma_start(t[:], dram[:, :].rearrange("o (f p) -> p (o f)", p=128))
    return t


def _bias_row(nc, pool, dram, nfeat, tag):
    t = pool.tile([1, nfeat], bf, tag=tag, name=tag)
    nc.sync.dma_start(t[:], dram[:, :])
    return t


def _load_w(nc, pool, dram, rows, cols, tag):
    tiles = []
    for fi in range(rows // 128):
        t = pool.tile([128, cols], bf, tag=f"{tag}{fi}", name=f"{tag}{fi}")
        nc.sync.dma_start(t[:], dram[fi * 128 : (fi + 1) * 128, :])
        tiles.append(t)
    return tiles


def _linT(nc, ps_pool, dst, w_tiles, src, bias_row, ones_row, act, n_cols):
    """dst[fo][:, :n_cols] = act(sum_fi w[fi][:, fo].T @ src[fi] + bias[fo]).

    bias_row: [1, F_out] bf16 SBUF row; folded in as a K=1 matmul."""
    nfi = len(w_tiles)
    for fo in range(len(dst)):
        for ck in range((n_cols + 511) // 512):
            c0, c1 = ck * 512, min((ck + 1) * 512, n_cols)
            ps = ps_pool.tile([128, 512], f32, tag="ps", name="ps")
            if bias_row is not None:
                nc.tensor.matmul(
                    ps[:, : c1 - c0],
                    bias_row[:, fo * 128 : (fo + 1) * 128],
                    ones_row[:, : c1 - c0],
                    start=True, stop=False,
                )
            for fi in range(nfi):
                nc.tensor.matmul(
                    ps[:, : c1 - c0],
                    w_tiles[fi][:, fo * 128 : (fo + 1) * 128],
                    src[fi][:, c0:c1],
                    start=(bias_row is None and fi == 0),
                    stop=(fi == nfi - 1),
                )
            nc.scalar.activation(dst[fo][:, c0:c1], ps[:, : c1 - c0], act)


def build_A():
    """Embedding: h0^T full (bf16) + h0 local (fp32)."""
    nc = _new_nc()
    COPY = _act_fn("Copy", "Identity", "copy")
    featT = nc.dram_tensor("featT", [IN_DIM, N], bf, kind="ExternalInput")
    featTl = nc.dram_tensor("featTl", [IN_DIM, NL], bf, kind="ExternalInput")
    wemb_d = nc.dram_tensor("wemb", [IN_DIM, F], bf, kind="ExternalInput")
    bemb_d = nc.dram_tensor("bemb", [1, F], f32, kind="ExternalInput")
    bembr_d = nc.dram_tensor("bembr", [1, F], bf, kind="ExternalInput")
    h0T_d = nc.dram_tensor("h0T", [F, N], bf, kind="ExternalOutput")
    h0l_d = nc.dram_tensor("h0l", [F, NL], f32, kind="ExternalOutput")

    with tile.TileContext(nc) as tc:
        with (
            tc.tile_pool(name="sb", bufs=2) as sb,
            tc.tile_pool(name="st", bufs=3) as st,
            tc.tile_pool(name="psum", bufs=3, space="PSUM") as psp,
        ):
            wemb = _load_w(nc, sb, wemb_d, IN_DIM, F, "wemb")
            bemb = _bias_tile(nc, sb, bemb_d, F, "bemb")
            bembr = _bias_row(nc, sb, bembr_d, F, "bembr")
            ones_row = sb.tile([1, 512], bf, tag="ones_row", name="ones_row")
            nc.gpsimd.memset(ones_row[:], 1.0)
            for ck in range(N // 512):
                fts = []
                for fi in range(IN_DIM // 128):
                    t = st.tile([128, 512], bf, tag=f"ftf{fi % 4}", name=f"ftf{fi % 4}")
                    nc.sync.dma_start(
                        t[:], featT[fi * 128 : (fi + 1) * 128, ck * 512 : (ck + 1) * 512]
                    )
                    fts.append(t)
                for fo in range(FT):
                    ps = psp.tile([128, 512], f32, tag="ps", name="ps")
                    nc.tensor.matmul(
                        ps[:], bembr[:, fo * 128 : (fo + 1) * 128], ones_row[:],
                        start=True, stop=False,
                    )
                    for fi in range(IN_DIM // 128):
                        nc.tensor.matmul(
                            ps[:], wemb[fi][:, fo * 128 : (fo + 1) * 128], fts[fi][:],
                            start=False, stop=(fi == IN_DIM // 128 - 1),
                        )
                    ot = st.tile([128, 512], bf, tag="ot", name="ot")
                    nc.scalar.activation(ot[:], ps[:], COPY)
                    nc.sync.dma_start(
                        h0T_d[fo * 128 : (fo + 1) * 128, ck * 512 : (ck + 1) * 512], ot[:]
                    )
            ftl = []
            for fi in range(IN_DIM // 128):
                t = st.tile([128, NL], bf, tag=f"ftl{fi}", name=f"ftl{fi}")
                nc.sync.dma_start(t[:], featTl[fi * 128 : (fi + 1) * 128, :])
                ftl.append(t)
            for fo in range(FT):
                ps = psp.tile([128, 512], f32, tag="ps", name="ps")
                for fi in range(IN_DIM // 128):
                    nc.tensor.matmul(
                        ps[:], wemb[fi][:, fo * 128 : (fo + 1) * 128], ftl[fi][:],
                        start=(fi == 0), stop=(fi == IN_DIM // 128 - 1),
                    )
                ol = st.tile([128, NL], f32, tag="ol", name="ol")
                nc.vector.tensor_scalar(ol[:], ps[:], bemb[:, fo : fo + 1], None, op0=ADD)
                nc.sync.dma_start(h0l_d[fo * 128 : (fo + 1) * 128, :], ol[:])
    nc.compile()
    return nc


def build_B1():
    """Attention block: X = O-proj(attn(h)) + h, bn1 partial sums."""
    nc = _new_nc()
    EXP = _act_fn("Exp", "exp")
    COPY = _act_fn("Copy", "Identity", "copy")
    SQUARE = _act_fn("Square", "square")

    hT_d = nc.dram_tensor("hT", [F, N], bf, kind="ExternalInput")
    hl_d = nc.dram_tensor("hl", [F, NL], f32, kind="ExternalInput")
    AT_d = nc.dram_tensor("AT", [N, NL], bf, kind="ExternalInput")
    wq_d = nc.dram_tensor("wq", [F, F], bf, kind="ExternalInput")
    wk_d = nc.dram_tensor("wk", [F, F], bf, kind="ExternalInput")
    wv_d = nc.dram_tensor("wv", [F, F], bf, kind="ExternalInput")
    wo_d = nc.dram_tensor("wo", [F, F], bf, kind="ExternalInput")
    bq_d = nc.dram_tensor("bq", [1, F], bf, kind="ExternalInput")
    bk_d = nc.dram_tensor("bk", [1, F], bf, kind="ExternalInput")
    bo2_d = nc.dram_tensor("bo2", [1, F], f32, kind="ExternalInput")
    X_d = nc.dram_tensor("X", [F, NL], f32, kind="ExternalOutput")
    p1_d = nc.dram_tensor("p1", [128, 8], f32, kind="ExternalOutput")

    with tile.TileContext(nc) as tc:
        with (
            tc.tile_pool(name="pp", bufs=1) as pp,
            tc.tile_pool(name="wp", bufs=1) as wp,
            tc.tile_pool(name="st", bufs=3) as st,
            tc.tile_pool(name="ep", bufs=6) as epo,
            tc.tile_pool(name="psA", bufs=4, space="PSUM") as psA,
            tc.tile_pool(name="psB", bufs=2, space="PSUM") as psB,
            tc.tile_pool(name="psO", bufs=2, space="PSUM") as psOp,
        ):
            AT = [pp.tile([128, NL], bf, tag=f"AT{m}", name=f"AT{m}") for m in range(MB)]
            for m in range(MB):
                nc.sync.dma_start(AT[m][:], AT_d[m * 128 : (m + 1) * 128, :])
            hT = [pp.tile([128, N], bf, tag=f"hT{t}", name=f"hT{t}") for t in range(FT)]
            for t in range(FT):
                nc.sync.dma_start(hT[t][:], hT_d[t * 128 : (t + 1) * 128, :])
            hc = [pp.tile([128, NL], f32, tag=f"hc{t}", name=f"hc{t}") for t in range(FT)]
            hTl = [pp.tile([128, NL], bf, tag=f"hTl{t}", name=f"hTl{t}") for t in range(FT)]
            for t in range(FT):
                nc.sync.dma_start(hc[t][:], hl_d[t * 128 : (t + 1) * 128, :])
                nc.vector.tensor_copy(hTl[t][:], hc[t][:])
            KT = [pp.tile([128, N], bf, tag=f"KT{t}", name=f"KT{t}") for t in range(FT)]
            Vn = [pp.tile([128, H * (D + 1)], bf, tag=f"Vn{m}", name=f"Vn{m}") for m in range(MB)]
            QT = [pp.tile([128, NL], bf, tag=f"QT{t}", name=f"QT{t}") for t in range(FT)]
            OT = [pp.tile([128, NL], bf, tag=f"OT{t}", name=f"OT{t}") for t in range(FT)]
            ones64 = pp.tile([1, 64], bf, tag="ones64", name="ones64")
            nc.gpsimd.memset(ones64[:], 1.0)

            wq = _load_w(nc, wp, wq_d, F, F, "wq")
            wk = _load_w(nc, wp, wk_d, F, F, "wk")
            wv = _load_w(nc, wp, wv_d, F, F, "wv")
            wo = _load_w(nc, wp, wo_d, F, F, "wo")
            bq = _bias_row(nc, wp, bq_d, F, "bq")
            bk = _bias_row(nc, wp, bk_d, F, "bk")
            bo2 = _bias_tile(nc, wp, bo2_d, F, "bo2")
            ones_row = wp.tile([1, 512], bf, tag="ones_row", name="ones_row")
            nc.gpsimd.memset(ones_row[:], 1.0)

            _linT(nc, psB, KT, wk, hT, bk, ones_row, COPY, N)
            _linT(nc, psB, QT, wq, hTl, bq, ones_row, COPY, NL)
            for m in range(MB):
                nc.gpsimd.memset(Vn[m][:], 1.0)
                ps = psB.tile([128, 512], f32, tag="ps", name="ps")
                for fi in range(FT):
                    nc.tensor.matmul(
                        ps[:], hT[fi][:, m * 128 : (m + 1) * 128], wv[fi][:],
                        start=(fi == 0), stop=(fi == FT - 1),
                    )
                nc.vector.tensor_copy(
                    Vn[m][:].rearrange("p (h x) -> p (h x)", x=D + 1)
                    .rearrange("p (h x) -> p h x", x=D + 1)[:, :, :D],
                    ps[:].rearrange("p (h d) -> p h d", d=D),
                )

            for h in range(H):
                ft, po = h // 2, 64 * (h % 2)
                pO = psOp.tile([128, NL], f32, tag="pO", name="pO")
                epair = None
                for m in range(MB):
                    pS = psA.tile([128, NL], f32, tag="pS", name="pS")
                    nc.tensor.matmul(
                        pS[:],
                        KT[ft][po : po + 64, m * 128 : (m + 1) * 128],
                        QT[ft][po : po + 64, :],
                        start=True, stop=True,
                    )
                    if m % 2 == 0:
                        epair = epo.tile([128, 2 * NL], bf, tag="epair", name="epair")
                    nc.vector.tensor_tensor(
                        epair[:, (m % 2) * NL : (m % 2 + 1) * NL], pS[:], AT[m][:], op=MUL
                    )
                    if m % 2 == 1:
                        nc.scalar.activation(epair[:], epair[:], EXP)
                        for mm in (m - 1, m):
                            nc.tensor.matmul(
                                pO[: D + 1, :],
                                Vn[mm][:, h * (D + 1) : (h + 1) * (D + 1)],
                                epair[:, (mm % 2) * NL : (mm % 2 + 1) * NL],
                                start=(mm == 0), stop=(mm == MB - 1),
                            )
                rs = st.tile([1, NL], bf, tag="rs", name="rs")
                with nc.allow_low_precision(reason="softmax denom recip in bf16"):
                    nc.vector.reciprocal(rs[:], pO[D : D + 1, :])
                pR = psA.tile([128, NL], f32, tag="pS", name="pS")
                nc.tensor.matmul(pR[:64, :], ones64[:], rs[:], start=True, stop=True)
                rb = st.tile([64, NL], bf, tag="rb", name="rb")
                nc.vector.tensor_copy(rb[:], pR[:64, :])
                nc.vector.tensor_tensor(
                    OT[ft][po : po + 64, :], pO[:D, :], rb[:], op=MUL
                )

            part = st.tile([128, 8], f32, tag="part", name="part")
            sq = st.tile([128, NL], f32, tag="sq", name="sq")
            for fo in range(FT):
                nc.vector.tensor_scalar(
                    hc[fo][:], hc[fo][:], bo2[:, fo : fo + 1], None, op0=ADD
                )
                ps = psB.tile([128, 512], f32, tag="ps", name="ps")
                for fi in range(FT):
                    nc.tensor.matmul(
                        ps[:], wo[fi][:, fo * 128 : (fo + 1) * 128], OT[fi][:],
                        start=(fi == 0), stop=(fi == FT - 1),
                    )
                Xt = st.tile([128, NL], f32, tag="Xt", name="Xt")
                nc.vector.tensor_tensor(Xt[:], ps[:], hc[fo][:], op=ADD)
                nc.sync.dma_start(X_d[fo * 128 : (fo + 1) * 128, :], Xt[:])
                nc.vector.tensor_reduce(
                    part[:, 2 * fo : 2 * fo + 1], Xt[:], axis=AX, op=ADD
                )
                nc.scalar.activation(sq[:], Xt[:], SQUARE)
                nc.vector.tensor_reduce(
                    part[:, 2 * fo + 1 : 2 * fo + 2], sq[:], axis=AX, op=ADD
                )
            nc.sync.dma_start(p1_d[:, :], part[:])
    nc.compile()
    return nc


def build_B2():
    """bn1-apply + FFN + residual: y, bn2 partial sums."""
    nc = _new_nc()
    RELU = _act_fn("Relu", "relu")
    SQUARE = _act_fn("Square", "square")
    X_di = nc.dram_tensor("X", [F, NL], f32, kind="ExternalInput")
    s1_d = nc.dram_tensor("s1", [128, FT], f32, kind="ExternalInput")
    t1_d = nc.dram_tensor("t1", [128, FT], f32, kind="ExternalInput")
    w1_d = nc.dram_tensor("w1", [F, FF], bf, kind="ExternalInput")
    w2_d = nc.dram_tensor("w2", [FF, F], bf, kind="ExternalInput")
    b1f_d = nc.dram_tensor("b1f", [1, FF], bf, kind="ExternalInput")
    b2f_d = nc.dram_tensor("b2f", [1, F], f32, kind="ExternalInput")
    y_d = nc.dram_tensor("y", [F, NL], f32, kind="ExternalOutput")
    p2_d = nc.dram_tensor("p2", [128, 8], f32, kind="ExternalOutput")

    with tile.TileContext(nc) as tc:
        with (
            tc.tile_pool(name="sb", bufs=1) as sb,
            tc.tile_pool(name="st", bufs=3) as st,
            tc.tile_pool(name="psum", bufs=4, space="PSUM") as psp,
        ):
            w1 = _load_w(nc, sb, w1_d, F, FF, "w1")
            w2 = _load_w(nc, sb, w2_d, FF, F, "w2")
            b1f = _bias_row(nc, sb, b1f_d, FF, "b1f")
            b2f = _bias_tile(nc, sb, b2f_d, F, "b2f")
            ones_row = sb.tile([1, 512], bf, tag="ones_row", name="ones_row")
            nc.gpsimd.memset(ones_row[:], 1.0)
            s1 = sb.tile([128, FT], f32, tag="s1", name="s1")
            t1 = sb.tile([128, FT], f32, tag="t1", name="t1")
            nc.sync.dma_start(s1[:], s1_d[:, :])
            nc.sync.dma_start(t1[:], t1_d[:, :])
            h1 = [sb.tile([128, NL], f32, tag=f"h1_{t}", name=f"h1_{t}") for t in range(FT)]
            h1b = [sb.tile([128, NL], bf, tag=f"h1b{t}", name=f"h1b{t}") for t in range(FT)]
            for fo in range(FT):
                Xt = st.tile([128, NL], f32, tag="Xt", name="Xt")
                nc.sync.dma_start(Xt[:], X_di[fo * 128 : (fo + 1) * 128, :])
                nc.vector.tensor_scalar(
                    h1[fo][:], Xt[:], s1[:, fo : fo + 1], t1[:, fo : fo + 1],
                    op0=MUL, op1=ADD,
                )
                nc.vector.tensor_copy(h1b[fo][:], h1[fo][:])
            r16 = [sb.tile([128, NL], bf, tag=f"r16_{g}", name=f"r16_{g}") for g in range(FF // 128)]
            _linT(nc, psp, r16, w1, h1b, b1f, ones_row, RELU, NL)
            part = st.tile([128, 8], f32, tag="part", name="part")
            sq = st.tile([128, NL], f32, tag="sq", name="sq")
            for fo in range(FT):
                nc.vector.tensor_scalar(
                    h1[fo][:], h1[fo][:], b2f[:, fo : fo + 1], None, op0=ADD
                )
                ps = psp.tile([128, 512], f32, tag="ps", name="ps")
                for gi in range(FF // 128):
                    nc.tensor.matmul(
                        ps[:], w2[gi][:, fo * 128 : (fo + 1) * 128], r16[gi][:],
                        start=(gi == 0), stop=(gi == FF // 128 - 1),
                    )
                yt = st.tile([128, NL], f32, tag="yt", name="yt")
                nc.vector.tensor_tensor(yt[:], ps[:], h1[fo][:], op=ADD)
                nc.sync.dma_start(y_d[fo * 128 : (fo + 1) * 128, :], yt[:])
                SQ = _act_fn("Square", "square")
                nc.vector.tensor_reduce(
                    part[:, 2 * fo : 2 * fo + 1], yt[:], axis=AX, op=ADD
                )
                nc.scalar.activation(sq[:], yt[:], SQ)
                nc.vector.tensor_reduce(
                    part[:, 2 * fo + 1 : 2 * fo + 2], sq[:], axis=AX, op=ADD
                )
            nc.sync.dma_start(p2_d[:, :], part[:])
    nc.compile()
    return nc


def build_C():
    """Prediction head on local rows."""
    nc = _new_nc()
    RELU = _act_fn("Relu", "relu")
    hl_d = nc.dram_tensor("hl", [F, NL], f32, kind="ExternalInput")
    wp1_d = nc.dram_tensor("wp1", [F, F // 2], bf, kind="ExternalInput")
    bp1_d = nc.dram_tensor("bp1", [1, F // 2], bf, kind="ExternalInput")
    wp2_d = nc.dram_tensor("wp2", [F // 2, NCL], bf, kind="ExternalInput")
    bp2_d = nc.dram_tensor("bp2", [1, NCL], f32, kind="ExternalInput")
    ident_d = nc.dram_tensor("ident", [128, 128], f32, kind="ExternalInput")
    out_d = nc.dram_tensor("out", [NL, NCL], f32, kind="ExternalOutput")
    with tile.TileContext(nc) as tc:
        with (
            tc.tile_pool(name="sb", bufs=1) as sb,
            tc.tile_pool(name="st", bufs=3) as st,
            tc.tile_pool(name="psum", bufs=3, space="PSUM") as psp,
        ):
            hTl = [sb.tile([128, NL], bf, tag=f"hTl{t}", name=f"hTl{t}") for t in range(FT)]
            for t in range(FT):
                hf = st.tile([128, NL], f32, tag="hf", name="hf")
                nc.sync.dma_start(hf[:], hl_d[t * 128 : (t + 1) * 128, :])
                nc.vector.tensor_copy(hTl[t][:], hf[:])
            wp1 = _load_w(nc, sb, wp1_d, F, F // 2, "wp1")
            bp1 = _bias_row(nc, sb, bp1_d, F // 2, "bp1")
            ones_row = sb.tile([1, 512], bf, tag="ones_row", name="ones_row")
            nc.gpsimd.memset(ones_row[:], 1.0)
            wp2 = _load_w(nc, sb, wp2_d, F // 2, NCL, "wp2")
            bp2 = sb.tile([NCL, 1], f32, tag="bp2", name="bp2")
            nc.sync.dma_start(bp2[:], bp2_d[:, :].rearrange("o p -> p o"))
            ident = sb.tile([128, 128], f32, tag="ident", name="ident")
            nc.sync.dma_start(ident[:], ident_d[:, :])
            p1 = [sb.tile([128, NL], bf, tag=f"p1_{t}", name=f"p1_{t}") for t in range(2)]
            _linT(nc, psp, p1, wp1, hTl, bp1, ones_row, RELU, NL)
            ps = psp.tile([NCL, NL], f32, tag="pred", name="pred", bufs=1)
            for pi in range(2):
                nc.tensor.matmul(
                    ps[:], wp2[pi][:, :], p1[pi][:], start=(pi == 0), stop=(pi == 1)
                )
            outT = st.tile([NCL, NL], f32, tag="outT", name="outT")
            nc.vector.tensor_scalar(outT[:], ps[:], bp2[:, :], None, op0=ADD)
            for ck in range(NL // 128):
                pt = psp.tile([128, NCL], f32, tag="pt", name="pt", bufs=2)
                nc.tensor.matmul(
                    pt[:], outT[:, ck * 128 : (ck + 1) * 128], ident[:NCL, :NCL],
                    start=True, stop=True, is_transpose=True,
                )
                ot = st.tile([128, NCL], f32, tag="ot", name="ot")
                nc.vector.tensor_copy(ot[:], pt[:])
                nc.sync.dma_start(out_d[ck * 128 : (ck + 1) * 128, :], ot[:])
    nc.compile()
    return nc


_CACHE = {}


def _get(name, builder):
    if name not in _CACHE:
        _CACHE[name] = builder()
    return _CACHE[name]


def _run(nc, in_maps, trace=False):
    return run_bass_kernel_spmd(
        nc, in_maps, core_ids=list(range(NCORES)), trace=trace
    )


def _bn_host(parts, g, b):
    """parts: list of [128, 8] per core -> s,t as [128, 4] per-partition."""
    tot = np.sum(np.stack(parts, 0), axis=0)  # [128, 8]
    s = tot.reshape(128, 4, 2)[:, :, 0]  # sums,  feature f = fo*128 + p
    ss = tot.reshape(128, 4, 2)[:, :, 1]
    mu = s / N
    var = ss / N - mu * mu
    gpf = g.reshape(4, 128).T  # [p, fo]
    bpf = b.reshape(4, 128).T
    sc = gpf / np.sqrt(var + EPS)
    sh = bpf - mu * sc
    return sc.astype(np.float32), sh.astype(np.float32), mu, var


def _bn_apply_full(yT_full, sc, sh):
    """yT_full [512, 4096]; sc/sh [128, 4] (p, fo) -> per-feature."""
    scf = sc.T.reshape(F, 1)  # feature f = fo*128+p -> row-major (fo, p)
    shf = sh.T.reshape(F, 1)
    return yT_full * scf + shf


def kernel(A, features, params, trace=False, timing=None):
    A = np.asarray(A, np.float32)
    features = np.asarray(features, np.float32)
    perm = np.array([d * H + h for h in range(H) for d in range(D)])

    W = lambda p: np.asarray(p["w"], np.float32)
    Bv = lambda p: np.asarray(p["b"], np.float32)

    featT = np.ascontiguousarray(features.T).astype(bf16)
    AT = np.ascontiguousarray(A.T).astype(bf16)

    def _t(res):
        if timing is not None and res.exec_time_ns:
            timing.append(res.exec_time_ns)

    # ---- A: embedding ----
    base = {
        "featT": featT,
        "wemb": W(params["emb"]).astype(bf16),
        "bemb": Bv(params["emb"]).reshape(1, F).astype(np.float32),
        "bembr": Bv(params["emb"]).reshape(1, F).astype(bf16),
    }
    maps = []
    for c in range(NCORES):
        m = dict(base)
        m["featTl"] = np.ascontiguousarray(featT[:, c * NL : (c + 1) * NL])
        maps.append(m)
    resA = _run(_get("A", build_A), maps, trace)
    _t(resA)
    hT_full = resA.results[0]["h0T"]  # bf16 [F, N]
    hl = [resA.results[c]["h0l"] for c in range(NCORES)]  # f32 [F, NL]

    ncB1 = _get("B1", build_B1)
    ncB2 = _get("B2", build_B2)
    for l, lp in enumerate(params["layers"]):
        wq = (W(lp["q"])[:, perm] * SCALING).astype(bf16)
        bq = (Bv(lp["q"])[perm] * SCALING).reshape(1, F).astype(bf16)
        wk = W(lp["k"])[:, perm].astype(bf16)
        bk = Bv(lp["k"])[perm].reshape(1, F).astype(bf16)
        wv = W(lp["v"])[:, perm].astype(bf16)
        wo = W(lp["o"])[perm, :].astype(bf16)
        bo2 = (Bv(lp["v"]) @ W(lp["o"]) + Bv(lp["o"])).reshape(1, F).astype(np.float32)
        maps = []
        for c in range(NCORES):
            maps.append({
                "hT": np.ascontiguousarray(hT_full),
                "hl": np.ascontiguousarray(hl[c]),
                "AT": np.ascontiguousarray(AT[:, c * NL : (c + 1) * NL]),
                "wq": wq, "bq": bq, "wk": wk, "bk": bk, "wv": wv,
                "wo": wo, "bo2": bo2,
            })
        r1 = _run(ncB1, maps, trace)
        _t(r1)
        Xs = [r1.results[c]["X"] for c in range(NCORES)]
        parts1 = [r1.results[c]["p1"] for c in range(NCORES)]
        s1, t1, _, _ = _bn_host(parts1, np.asarray(lp["g1"], np.float32),
                                np.asarray(lp["b1"], np.float32))
        maps = []
        w1 = W(lp["ffn1"]).astype(bf16)
        w2 = W(lp["ffn2"]).astype(bf16)
        b1f = Bv(lp["ffn1"]).reshape(1, FF).astype(bf16)
        b2f = Bv(lp["ffn2"]).reshape(1, F).astype(np.float32)
        for c in range(NCORES):
            maps.append({
                "X": Xs[c], "s1": s1, "t1": t1,
                "w1": w1, "w2": w2, "b1f": b1f, "b2f": b2f,
            })
        r2 = _run(ncB2, maps, trace)
        _t(r2)
        ys = [r2.results[c]["y"] for c in range(NCORES)]
        parts2 = [r2.results[c]["p2"] for c in range(NCORES)]
        s2, t2, _, _ = _bn_host(parts2, np.asarray(lp["g2"], np.float32),
                                np.asarray(lp["b2"], np.float32))
        yT_full = np.concatenate(ys, axis=1)  # [F, N] f32
        hT_next = _bn_apply_full(yT_full.astype(np.float32), s2, t2)
        hT_full = hT_next.astype(bf16)
        hl = [np.ascontiguousarray(hT_next[:, c * NL : (c + 1) * NL]).astype(np.float32)
              for c in range(NCORES)]

    # ---- C: prediction head ----
    maps = []
    basec = {
        "wp1": W(params["pred1"]).astype(bf16),
        "bp1": Bv(params["pred1"]).reshape(1, F // 2).astype(bf16),
        "wp2": W(params["pred2"]).astype(bf16),
        "bp2": Bv(params["pred2"]).reshape(1, NCL).astype(np.float32),
        "ident": np.eye(128, dtype=np.float32),
    }
    for c in range(NCORES):
        m = dict(basec)
        m["hl"] = hl[c]
        maps.append(m)
    resC = _run(_get("C", build_C), maps, trace)
    _t(resC)
    out = np.concatenate([resC.results[c]["out"] for c in range(NCORES)], axis=0)
    return np.asarray(out, np.float32)


# revision 9
# speedup vs baseline: 1.0731x; 1.0441x over previous
"""Graph-transformer (nn_GTModel) Trainium2 kernel, 8-core SPMD, multi-launch.

Device work is split into 4 NEFFs (collectives are unavailable on this
runtime, so the tiny cross-core exchanges go through the host):
  A  : embedding       -> h0^T (full, bf16) + local h0 (fp32)
  B1 : attention block -> X = O-proj(attn) + h (local, fp32) + bn1 partial sums
  B2 : bn1-apply + FFN -> y = h1 + ffn(h1)  (local, fp32) + bn2 partial sums
  C  : prediction head -> out rows (local)
Host between launches: sum 4KB batchnorm partials, compute scale/shift,
apply the bn2 affine to assemble the next layer's full hidden state, and
reshard. All matmuls / softmax / elementwise heavy work is on device.

Layout/tricks:
- Feature-major activations hT [512 f, n]; node rows sharded 8 ways.
- Host permutes Wq/Wk/Wv cols (Wo rows) so heads are contiguous;
  attention scale folded into Wq/bq; V bias folded into O-proj bias.
- Scores computed transposed (S^T [m, n]): softmax denominator comes
  free from an appended ones-column on node-major V in the attn@V matmul.
- bf16 for TensorE-facing tensors, fp32 accumulation/residual/stats.
"""

import numpy as np
import ml_dtypes

import concourse.bacc as bacc
import concourse.mybir as mybir
import concourse.tile as tile
from concourse.bass_utils import run_bass_kernel_spmd

NCORES = 8
N = 4096
NL = N // NCORES
F = 512
H = 8
D = 64
FF = 1024
IN_DIM = 1024
NCL = 20
EPS = 1e-5
SCALING = float(F) ** -0.5
FT = F // 128
MB = N // 128

bf16 = ml_dtypes.bfloat16
f32 = mybir.dt.float32
bf = mybir.dt.bfloat16

AX = mybir.AxisListType.X
MUL = mybir.AluOpType.mult
ADD = mybir.AluOpType.add
SUB = mybir.AluOpType.subtract


def _act_fn(*names):
    t = mybir.ActivationFunctionType
    for n in names:
        if hasattr(t, n):
            return getattr(t, n)
    raise AttributeError(names)


def _new_nc():
    return bacc.Bacc("TRN2", target_bir_lowering=False, debug=False, num_devices=NCORES)


def _bias_tile(nc, pool, dram, nfeat, tag):
    """Load a [1, nfeat] DRAM vector as per-partition [128, nfeat//128]."""
    t = pool.tile([128, nfeat // 128], f32, tag=tag)
    nc.sync.dma_start(t[:], dram[:, :].rearrange("o (f p) -> p (o f)", p=128))
    return t


def _bias_row(nc, pool, dram, nfeat, tag):
    t = pool.tile([1, nfeat], bf, tag=tag, name=tag)
    nc.sync.dma_start(t[:], dram[:, :])
    return t


def _load_w(nc, pool, dram, rows, cols, tag):
    tiles = []
    for fi in range(rows // 128):
        t = pool.tile([128, cols], bf, tag=f"{tag}{fi}", name=f"{tag}{fi}")
        nc.sync.dma_start(t[:], dram[fi * 128 : (fi + 1) * 128, :])
        tiles.append(t)
    return tiles


def _linT(nc, ps_pool, dst, w_tiles, src, bias_row, ones_row, act, n_cols):
    """dst[fo][:, :n_cols] = act(sum_fi w[fi][:, fo].T @ src[fi] + bias[fo]).

    bias_row: [1, F_out] bf16 SBUF row; folded in as a K=1 matmul."""
    nfi = len(w_tiles)
    for fo in range(len(dst)):
        for ck in range((n_cols + 511) // 512):
            c0, c1 = ck * 512, min((ck + 1) * 512, n_cols)
            ps = ps_pool.tile([128, 512], f32, tag="ps", name="ps")
            if bias_row is not None:
                nc.tensor.matmul(
                    ps[:, : c1 - c0],
                    bias_row[:, fo * 128 : (fo + 1) * 128],
                    ones_row[:, : c1 - c0],
                    start=True, stop=False,
                )
            for fi in range(nfi):
                nc.tensor.matmul(
                    ps[:, : c1 - c0],
                    w_tiles[fi][:, fo * 128 : (fo + 1) * 128],
                    src[fi][:, c0:c1],
                    start=(bias_row is None and fi == 0),
                    stop=(fi == nfi - 1),
                )
            nc.scalar.activation(dst[fo][:, c0:c1], ps[:, : c1 - c0], act)


def build_A():
    """Embedding: h0^T full (bf16) + h0 local (fp32)."""
    nc = _new_nc()
    COPY = _act_fn("Copy", "Identity", "copy")
    featT = nc.dram_tensor("featT", [IN_DIM, N], bf, kind="ExternalInput")
    featTl = nc.dram_tensor("featTl", [IN_DIM, NL], bf, kind="ExternalInput")
    wemb_d = nc.dram_tensor("wemb", [IN_DIM, F], bf, kind="ExternalInput")
    bemb_d = nc.dram_tensor("bemb", [1, F], f32, kind="ExternalInput")
    bembr_d = nc.dram_tensor("bembr", [1, F], bf, kind="ExternalInput")
    h0T_d = nc.dram_tensor("h0T", [F, N], bf, kind="ExternalOutput")
    h0l_d = nc.dram_tensor("h0l", [F, NL], f32, kind="ExternalOutput")

    with tile.TileContext(nc) as tc:
        with (
            tc.tile_pool(name="sb", bufs=2) as sb,
            tc.tile_pool(name="st", bufs=3) as st,
            tc.tile_pool(name="psum", bufs=3, space="PSUM") as psp,
        ):
            wemb = _load_w(nc, sb, wemb_d, IN_DIM, F, "wemb")
            bemb = _bias_tile(nc, sb, bemb_d, F, "bemb")
            bembr = _bias_row(nc, sb, bembr_d, F, "bembr")
            ones_row = sb.tile([1, 512], bf, tag="ones_row", name="ones_row")
            nc.gpsimd.memset(ones_row[:], 1.0)
            for ck in range(N // 512):
                fts = []
                for fi in range(IN_DIM // 128):
                    t = st.tile([128, 512], bf, tag=f"ftf{fi % 4}", name=f"ftf{fi % 4}")
                    nc.sync.dma_start(
                        t[:], featT[fi * 128 : (fi + 1) * 128, ck * 512 : (ck + 1) * 512]
                    )
                    fts.append(t)
                for fo in range(FT):
                    ps = psp.tile([128, 512], f32, tag="ps", name="ps")
                    nc.tensor.matmul(
                        ps[:], bembr[:, fo * 128 : (fo + 1) * 128], ones_row[:],
                        start=True, stop=False,
                    )
                    for fi in range(IN_DIM // 128):
                        nc.tensor.matmul(
                            ps[:], wemb[fi][:, fo * 128 : (fo + 1) * 128], fts[fi][:],
                            start=False, stop=(fi == IN_DIM // 128 - 1),
                        )
                    ot = st.tile([128, 512], bf, tag="ot", name="ot")
                    nc.scalar.activation(ot[:], ps[:], COPY)
                    nc.sync.dma_start(
                        h0T_d[fo * 128 : (fo + 1) * 128, ck * 512 : (ck + 1) * 512], ot[:]
                    )
            ftl = []
            for fi in range(IN_DIM // 128):
                t = st.tile([128, NL], bf, tag=f"ftl{fi}", name=f"ftl{fi}")
                nc.sync.dma_start(t[:], featTl[fi * 128 : (fi + 1) * 128, :])
                ftl.append(t)
            for fo in range(FT):
                ps = psp.tile([128, 512], f32, tag="ps", name="ps")
                for fi in range(IN_DIM // 128):
                    nc.tensor.matmul(
                        ps[:], wemb[fi][:, fo * 128 : (fo + 1) * 128], ftl[fi][:],
                        start=(fi == 0), stop=(fi == IN_DIM // 128 - 1),
                    )
                ol = st.tile([128, NL], f32, tag="ol", name="ol")
                nc.vector.tensor_scalar(ol[:], ps[:], bemb[:, fo : fo + 1], None, op0=ADD)
                nc.sync.dma_start(h0l_d[fo * 128 : (fo + 1) * 128, :], ol[:])
    nc.compile()
    return nc


def build_B1():
    """Attention block: X = O-proj(attn(h)) + h, bn1 partial sums."""
    nc = _new_nc()
    EXP = _act_fn("Exp", "exp")
    COPY = _act_fn("Copy", "Identity", "copy")
    SQUARE = _act_fn("Square", "square")

    hT_d = nc.dram_tensor("hT", [F, N], bf, kind="ExternalInput")
    hl_d = nc.dram_tensor("hl", [F, NL], f32, kind="ExternalInput")
    AT_d = nc.dram_tensor("AT", [N, NL], bf, kind="ExternalInput")
    wq_d = nc.dram_tensor("wq", [F, F], bf, kind="ExternalInput")
    wk_d = nc.dram_tensor("wk", [F, F], bf, kind="ExternalInput")
    wv_d = nc.dram_tensor("wv", [F, F], bf, kind="ExternalInput")
    wo_d = nc.dram_tensor("wo", [F, F], bf, kind="ExternalInput")
    bq_d = nc.dram_tensor("bq", [1, F], bf, kind="ExternalInput")
    bk_d = nc.dram_tensor("bk", [1, F], bf, kind="ExternalInput")
    bo2_d = nc.dram_tensor("bo2", [1, F], f32, kind="ExternalInput")
    X_d = nc.dram_tensor("X", [F, NL], f32, kind="ExternalOutput")
    p1_d = nc.dram_tensor("p1", [128, 8], f32, kind="ExternalOutput")

    with tile.TileContext(nc) as tc:
        with (
            tc.tile_pool(name="pp", bufs=1) as pp,
            tc.tile_pool(name="wp", bufs=1) as wp,
            tc.tile_pool(name="st", bufs=3) as st,
            tc.tile_pool(name="ep", bufs=6) as epo,
            tc.tile_pool(name="psA", bufs=4, space="PSUM") as psA,
            tc.tile_pool(name="psB", bufs=2, space="PSUM") as psB,
            tc.tile_pool(name="psO", bufs=2, space="PSUM") as psOp,
        ):
            AT = [pp.tile([128, NL], bf, tag=f"AT{m}", name=f"AT{m}") for m in range(MB)]
            hT = [pp.tile([128, N], bf, tag=f"hT{t}", name=f"hT{t}") for t in range(FT)]
            for t in range(FT):
                nc.sync.dma_start(hT[t][:], hT_d[t * 128 : (t + 1) * 128, :])
            hc = [pp.tile([128, NL], f32, tag=f"hc{t}", name=f"hc{t}") for t in range(FT)]
            hTl = [pp.tile([128, NL], bf, tag=f"hTl{t}", name=f"hTl{t}") for t in range(FT)]
            for t in range(FT):
                nc.sync.dma_start(hc[t][:], hl_d[t * 128 : (t + 1) * 128, :])
                nc.vector.tensor_copy(hTl[t][:], hc[t][:])
            KT = [pp.tile([128, N], bf, tag=f"KT{t}", name=f"KT{t}") for t in range(FT)]
            Vn = [pp.tile([128, H * (D + 1)], bf, tag=f"Vn{m}", name=f"Vn{m}") for m in range(MB)]
            QT = [pp.tile([128, NL], bf, tag=f"QT{t}", name=f"QT{t}") for t in range(FT)]
            OT = [pp.tile([128, NL], bf, tag=f"OT{t}", name=f"OT{t}") for t in range(FT)]
            ones64 = pp.tile([1, 64], bf, tag="ones64", name="ones64")
            nc.gpsimd.memset(ones64[:], 1.0)

            wq = _load_w(nc, wp, wq_d, F, F, "wq")
            wk = _load_w(nc, wp, wk_d, F, F, "wk")
            wv = _load_w(nc, wp, wv_d, F, F, "wv")
            wo = _load_w(nc, wp, wo_d, F, F, "wo")
            bq = _bias_row(nc, wp, bq_d, F, "bq")
            bk = _bias_row(nc, wp, bk_d, F, "bk")
            bo2 = _bias_tile(nc, wp, bo2_d, F, "bo2")
            ones_row = wp.tile([1, 512], bf, tag="ones_row", name="ones_row")
            nc.gpsimd.memset(ones_row[:], 1.0)

            _linT(nc, psB, KT, wk, hT, bk, ones_row, COPY, N)
            _linT(nc, psB, QT, wq, hTl, bq, ones_row, COPY, NL)
            for m in range(MB):
                nc.sync.dma_start(AT[m][:], AT_d[m * 128 : (m + 1) * 128, :])
                nc.gpsimd.memset(Vn[m][:], 1.0)
                ps = psB.tile([128, 512], f32, tag="ps", name="ps")
                for fi in range(FT):
                    nc.tensor.matmul(
                        ps[:], hT[fi][:, m * 128 : (m + 1) * 128], wv[fi][:],
                        start=(fi == 0), stop=(fi == FT - 1),
                    )
                nc.vector.tensor_copy(
                    Vn[m][:].rearrange("p (h x) -> p (h x)", x=D + 1)
                    .rearrange("p (h x) -> p h x", x=D + 1)[:, :, :D],
                    ps[:].rearrange("p (h d) -> p h d", d=D),
                )

            for h in range(H):
                ft, po = h // 2, 64 * (h % 2)
                pO = psOp.tile([128, NL], f32, tag="pO", name="pO")
                epair = None
                for m in range(MB):
                    pS = psA.tile([128, NL], f32, tag="pS", name="pS")
                    nc.tensor.matmul(
                        pS[:],
                        KT[ft][po : po + 64, m * 128 : (m + 1) * 128],
                        QT[ft][po : po + 64, :],
                        start=True, stop=True,
                    )
                    if m % 2 == 0:
                        epair = epo.tile([128, 2 * NL], bf, tag="epair", name="epair")
                    nc.vector.tensor_tensor(
                        epair[:, (m % 2) * NL : (m % 2 + 1) * NL], pS[:], AT[m][:], op=MUL
                    )
                    if m % 2 == 1:
                        nc.scalar.activation(epair[:], epair[:], EXP)
                        for mm in (m - 1, m):
                            nc.tensor.matmul(
                                pO[: D + 1, :],
                                Vn[mm][:, h * (D + 1) : (h + 1) * (D + 1)],
                                epair[:, (mm % 2) * NL : (mm % 2 + 1) * NL],
                                start=(mm == 0), stop=(mm == MB - 1),
                            )
                rs = st.tile([1, NL], bf, tag="rs", name="rs")
                with nc.allow_low_precision(reason="softmax denom recip in bf16"):
                    nc.vector.reciprocal(rs[:], pO[D : D + 1, :])
                pR = psA.tile([128, NL], f32, tag="pS", name="pS")
                nc.tensor.matmul(pR[:64, :], ones64[:], rs[:], start=True, stop=True)
                rb = st.tile([64, NL], bf, tag="rb", name="rb")
                nc.vector.tensor_copy(rb[:], pR[:64, :])
                nc.vector.tensor_tensor(
                    OT[ft][po : po + 64, :], pO[:D, :], rb[:], op=MUL
                )

            part = st.tile([128, 8], f32, tag="part", name="part")
            sq = st.tile([128, NL], f32, tag="sq", name="sq")
            for fo in range(FT):
                nc.vector.tensor_scalar(
                    hc[fo][:], hc[fo][:], bo2[:, fo : fo + 1], None, op0=ADD
                )
                ps = psB.tile([128, 512], f32, tag="ps", name="ps")
                for fi in range(FT):
                    nc.tensor.matmul(
                        ps[:], wo[fi][:, fo * 128 : (fo + 1) * 128], OT[fi][:],
                        start=(fi == 0), stop=(fi == FT - 1),
                    )
                Xt = st.tile([128, NL], f32, tag="Xt", name="Xt")
                nc.vector.tensor_tensor(Xt[:], ps[:], hc[fo][:], op=ADD)
                nc.sync.dma_start(X_d[fo * 128 : (fo + 1) * 128, :], Xt[:])
                nc.vector.tensor_reduce(
                    part[:, 2 * fo : 2 * fo + 1], Xt[:], axis=AX, op=ADD
                )
                nc.scalar.activation(sq[:], Xt[:], SQUARE)
                nc.vector.tensor_reduce(
                    part[:, 2 * fo + 1 : 2 * fo + 2], sq[:], axis=AX, op=ADD
                )
            nc.sync.dma_start(p1_d[:, :], part[:])
    nc.compile()
    return nc


def build_B2():
    """bn1-apply + FFN + residual: y, bn2 partial sums."""
    nc = _new_nc()
    RELU = _act_fn("Relu", "relu")
    SQUARE = _act_fn("Square", "square")
    X_di = nc.dram_tensor("X", [F, NL], f32, kind="ExternalInput")
    s1_d = nc.dram_tensor("s1", [128, FT], f32, kind="ExternalInput")
    t1_d = nc.dram_tensor("t1", [128, FT], f32, kind="ExternalInput")
    w1_d = nc.dram_tensor("w1", [F, FF], bf, kind="ExternalInput")
    w2_d = nc.dram_tensor("w2", [FF, F], bf, kind="ExternalInput")
    b1f_d = nc.dram_tensor("b1f", [1, FF], bf, kind="ExternalInput")
    b2f_d = nc.dram_tensor("b2f", [1, F], f32, kind="ExternalInput")
    y_d = nc.dram_tensor("y", [F, NL], f32, kind="ExternalOutput")
    p2_d = nc.dram_tensor("p2", [128, 8], f32, kind="ExternalOutput")

    with tile.TileContext(nc) as tc:
        with (
            tc.tile_pool(name="sb", bufs=1) as sb,
            tc.tile_pool(name="st", bufs=3) as st,
            tc.tile_pool(name="psum", bufs=4, space="PSUM") as psp,
        ):
            w1 = _load_w(nc, sb, w1_d, F, FF, "w1")
            w2 = _load_w(nc, sb, w2_d, FF, F, "w2")
            b1f = _bias_row(nc, sb, b1f_d, FF, "b1f")
            b2f = _bias_tile(nc, sb, b2f_d, F, "b2f")
            ones_row = sb.tile([1, 512], bf, tag="ones_row", name="ones_row")
            nc.gpsimd.memset(ones_row[:], 1.0)
            s1 = sb.tile([128, FT], f32, tag="s1", name="s1")
            t1 = sb.tile([128, FT], f32, tag="t1", name="t1")
            nc.sync.dma_start(s1[:], s1_d[:, :])
            nc.sync.dma_start(t1[:], t1_d[:, :])
            h1 = [sb.tile([128, NL], f32, tag=f"h1_{t}", name=f"h1_{t}") for t in range(FT)]
            h1b = [sb.tile([128, NL], bf, tag=f"h1b{t}", name=f"h1b{t}") for t in range(FT)]
            for fo in range(FT):
                Xt = st.tile([128, NL], f32, tag="Xt", name="Xt")
                nc.sync.dma_start(Xt[:], X_di[fo * 128 : (fo + 1) * 128, :])
                nc.vector.tensor_scalar(
                    h1[fo][:], Xt[:], s1[:, fo : fo + 1], t1[:, fo : fo + 1],
                    op0=MUL, op1=ADD,
                )
                nc.vector.tensor_copy(h1b[fo][:], h1[fo][:])
            r16 = [sb.tile([128, NL], bf, tag=f"r16_{g}", name=f"r16_{g}") for g in range(FF // 128)]
            _linT(nc, psp, r16, w1, h1b, b1f, ones_row, RELU, NL)
            part = st.tile([128, 8], f32, tag="part", name="part")
            sq = st.tile([128, NL], f32, tag="sq", name="sq")
            for fo in range(FT):
                nc.vector.tensor_scalar(
                    h1[fo][:], h1[fo][:], b2f[:, fo : fo + 1], None, op0=ADD
                )
                ps = psp.tile([128, 512], f32, tag="ps", name="ps")
                for gi in range(FF // 128):
                    nc.tensor.matmul(
                        ps[:], w2[gi][:, fo * 128 : (fo + 1) * 128], r16[gi][:],
                        start=(gi == 0), stop=(gi == FF // 128 - 1),
                    )
                yt = st.tile([128, NL], f32, tag="yt", name="yt")
                nc.vector.tensor_tensor(yt[:], ps[:], h1[fo][:], op=ADD)
                nc.sync.dma_start(y_d[fo * 128 : (fo + 1) * 128, :], yt[:])
                SQ = _act_fn("Square", "square")
                nc.vector.tensor_reduce(
                    part[:, 2 * fo : 2 * fo + 1], yt[:], axis=AX, op=ADD
                )
                nc.scalar.activation(sq[:], yt[:], SQ)
                nc.vector.tensor_reduce(
                    part[:, 2 * fo + 1 : 2 * fo + 2], sq[:], axis=AX, op=ADD
                )
            nc.sync.dma_start(p2_d[:, :], part[:])
    nc.compile()
    return nc


def build_C():
    """Prediction head on local rows."""
    nc = _new_nc()
    RELU = _act_fn("Relu", "relu")
    hl_d = nc.dram_tensor("hl", [F, NL], f32, kind="ExternalInput")
    wp1_d = nc.dram_tensor("wp1", [F, F // 2], bf, kind="ExternalInput")
    bp1_d = nc.dram_tensor("bp1", [1, F // 2], bf, kind="ExternalInput")
    wp2_d = nc.dram_tensor("wp2", [F // 2, NCL], bf, kind="ExternalInput")
    bp2_d = nc.dram_tensor("bp2", [1, NCL], f32, kind="ExternalInput")
    ident_d = nc.dram_tensor("ident", [128, 128], f32, kind="ExternalInput")
    out_d = nc.dram_tensor("out", [NL, NCL], f32, kind="ExternalOutput")
    with tile.TileContext(nc) as tc:
        with (
            tc.tile_pool(name="sb", bufs=1) as sb,
            tc.tile_pool(name="st", bufs=3) as st,
            tc.tile_pool(name="psum", bufs=3, space="PSUM") as psp,
        ):
            hTl = [sb.tile([128, NL], bf, tag=f"hTl{t}", name=f"hTl{t}") for t in range(FT)]
            for t in range(FT):
                hf = st.tile([128, NL], f32, tag="hf", name="hf")
                nc.sync.dma_start(hf[:], hl_d[t * 128 : (t + 1) * 128, :])
                nc.vector.tensor_copy(hTl[t][:], hf[:])
            wp1 = _load_w(nc, sb, wp1_d, F, F // 2, "wp1")
            bp1 = _bias_row(nc, sb, bp1_d, F // 2, "bp1")
            ones_row = sb.tile([1, 512], bf, tag="ones_row", name="ones_row")
            nc.gpsimd.memset(ones_row[:], 1.0)
            wp2 = _load_w(nc, sb, wp2_d, F // 2, NCL, "wp2")
            bp2 = sb.tile([NCL, 1], f32, tag="bp2", name="bp2")
            nc.sync.dma_start(bp2[:], bp2_d[:, :].rearrange("o p -> p o"))
            ident = sb.tile([128, 128], f32, tag="ident", name="ident")
            nc.sync.dma_start(ident[:], ident_d[:, :])
            p1 = [sb.tile([128, NL], bf, tag=f"p1_{t}", name=f"p1_{t}") for t in range(2)]
            _linT(nc, psp, p1, wp1, hTl, bp1, ones_row, RELU, NL)
            ps = psp.tile([NCL, NL], f32, tag="pred", name="pred", bufs=1)
            for pi in range(2):
                nc.tensor.matmul(
                    ps[:], wp2[pi][:, :], p1[pi][:], start=(pi == 0), stop=(pi == 1)
                )
            outT = st.tile([NCL, NL], f32, tag="outT", name="outT")
            nc.vector.tensor_scalar(outT[:], ps[:], bp2[:, :], None, op0=ADD)
            for ck in range(NL // 128):
                pt = psp.tile([128, NCL], f32, tag="pt", name="pt", bufs=2)
                nc.tensor.matmul(
                    pt[:], outT[:, ck * 128 : (ck + 1) * 128], ident[:NCL, :NCL],
                    start=True, stop=True, is_transpose=True,
                )
                ot = st.tile([128, NCL], f32, tag="ot", name="ot")
                nc.vector.tensor_copy(ot[:], pt[:])
                nc.sync.dma_start(out_d[ck * 128 : (ck + 1) * 128, :], ot[:])
    nc.compile()
    return nc


_CACHE = {}


def _get(name, builder):
    if name not in _CACHE:
        _CACHE[name] = builder()
    return _CACHE[name]


def _run(nc, in_maps, trace=False):
    return run_bass_kernel_spmd(
        nc, in_maps, core_ids=list(range(NCORES)), trace=trace
    )


def _bn_host(parts, g, b):
    """parts: list of [128, 8] per core -> s,t as [128, 4] per-partition."""
    tot = np.sum(np.stack(parts, 0), axis=0)  # [128, 8]
    s = tot.reshape(128, 4, 2)[:, :, 0]  # sums,  feature f = fo*128 + p
    ss = tot.reshape(128, 4, 2)[:, :, 1]
    mu = s / N
    var = ss / N - mu * mu
    gpf = g.reshape(4, 128).T  # [p, fo]
    bpf = b.reshape(4, 128).T
    sc = gpf / np.sqrt(var + EPS)
    sh = bpf - mu * sc
    return sc.astype(np.float32), sh.astype(np.float32), mu, var


def _bn_apply_full(yT_full, sc, sh):
    """yT_full [512, 4096]; sc/sh [128, 4] (p, fo) -> per-feature."""
    scf = sc.T.reshape(F, 1)  # feature f = fo*128+p -> row-major (fo, p)
    shf = sh.T.reshape(F, 1)
    return yT_full * scf + shf


def kernel(A, features, params, trace=False, timing=None):
    A = np.asarray(A, np.float32)
    features = np.asarray(features, np.float32)
    perm = np.array([d * H + h for h in range(H) for d in range(D)])

    W = lambda p: np.asarray(p["w"], np.float32)
    Bv = lambda p: np.asarray(p["b"], np.float32)

    featT = np.ascontiguousarray(features.T).astype(bf16)
    AT = np.ascontiguousarray(A.T).astype(bf16)

    def _t(res):
        if timing is not None and res.exec_time_ns:
            timing.append(res.exec_time_ns)

    # ---- A: embedding ----
    base = {
        "featT": featT,
        "wemb": W(params["emb"]).astype(bf16),
        "bemb": Bv(params["emb"]).reshape(1, F).astype(np.float32),
        "bembr": Bv(params["emb"]).reshape(1, F).astype(bf16),
    }
    maps = []
    for c in range(NCORES):
        m = dict(base)
        m["featTl"] = np.ascontiguousarray(featT[:, c * NL : (c + 1) * NL])
        maps.append(m)
    resA = _run(_get("A", build_A), maps, trace)
    _t(resA)
    hT_full = resA.results[0]["h0T"]  # bf16 [F, N]
    hl = [resA.results[c]["h0l"] for c in range(NCORES)]  # f32 [F, NL]

    ncB1 = _get("B1", build_B1)
    ncB2 = _get("B2", build_B2)
    for l, lp in enumerate(params["layers"]):
        wq = (W(lp["q"])[:, perm] * SCALING).astype(bf16)
        bq = (Bv(lp["q"])[perm] * SCALING).reshape(1, F).astype(bf16)
        wk = W(lp["k"])[:, perm].astype(bf16)
        bk = Bv(lp["k"])[perm].reshape(1, F).astype(bf16)
        wv = W(lp["v"])[:, perm].astype(bf16)
        wo = W(lp["o"])[perm, :].astype(bf16)
        bo2 = (Bv(lp["v"]) @ W(lp["o"]) + Bv(lp["o"])).reshape(1, F).astype(np.float32)
        maps = []
        for c in range(NCORES):
            maps.append({
                "hT": np.ascontiguousarray(hT_full),
                "hl": np.ascontiguousarray(hl[c]),
                "AT": np.ascontiguousarray(AT[:, c * NL : (c + 1) * NL]),
                "wq": wq, "bq": bq, "wk": wk, "bk": bk, "wv": wv,
                "wo": wo, "bo2": bo2,
            })
        r1 = _run(ncB1, maps, trace)
        _t(r1)
        Xs = [r1.results[c]["X"] for c in range(NCORES)]
        parts1 = [r1.results[c]["p1"] for c in range(NCORES)]
        s1, t1, _, _ = _bn_host(parts1, np.asarray(lp["g1"], np.float32),
                                np.asarray(lp["b1"], np.float32))
        maps = []
        w1 = W(lp["ffn1"]).astype(bf16)
        w2 = W(lp["ffn2"]).astype(bf16)
        b1f = Bv(lp["ffn1"]).reshape(1, FF).astype(bf16)
        b2f = Bv(lp["ffn2"]).reshape(1, F).astype(np.float32)
        for c in range(NCORES):
            maps.append({
                "X": Xs[c], "s1": s1, "t1": t1,
                "w1": w1, "w2": w2, "b1f": b1f, "b2f": b2f,
            })
        r2 = _run(ncB2, maps, trace)
        _t(r2)
        ys = [r2.results[c]["y"] for c in range(NCORES)]
        parts2 = [r2.results[c]["p2"] for c in range(NCORES)]
        s2, t2, _, _ = _bn_host(parts2, np.asarray(lp["g2"], np.float32),
                                np.asarray(lp["b2"], np.float32))
        yT_full = np.concatenate(ys, axis=1)  # [F, N] f32
        hT_next = _bn_apply_full(yT_full.astype(np.float32), s2, t2)
        hT_full = hT_next.astype(bf16)
        hl = [np.ascontiguousarray(hT_next[:, c * NL : (c + 1) * NL]).astype(np.float32)
              for c in range(NCORES)]

    # ---- C: prediction head ----
    maps = []
    basec = {
        "wp1": W(params["pred1"]).astype(bf16),
        "bp1": Bv(params["pred1"]).reshape(1, F // 2).astype(bf16),
        "wp2": W(params["pred2"]).astype(bf16),
        "bp2": Bv(params["pred2"]).reshape(1, NCL).astype(np.float32),
        "ident": np.eye(128, dtype=np.float32),
    }
    for c in range(NCORES):
        m = dict(basec)
        m["hl"] = hl[c]
        maps.append(m)
    resC = _run(_get("C", build_C), maps, trace)
    _t(resC)
    out = np.concatenate([resC.results[c]["out"] for c in range(NCORES)], axis=0)
    return np.asarray(out, np.float32)
